# revision 85
# baseline (speedup 1.0000x reference)
"""Trainium2 Bass kernel for nn_ImprintedModel (retrieval_knn).

Computes y[c, b] = max over the 32 proxies p of class c of
    (w1[p] / ||w1[p]||) . (data[b] / ||data[b]||)
for data [4096, 512], w1 [64000, 512] (2000 classes x 32 proxies),
output [2000, 4096] fp32.

Sharding: w1 rows (and hence classes) split across 8 cores (8000 rows =
250 classes per core); data replicated. Each core computes its 250
output rows for all 4096 batch columns; host concatenates/transposes.

Device algorithm (fp8 fast path; measured rel err ~1.77e-2 < 2e-2):
  * w rows are L2-normalized on device (ACT square+accum, half-block
    batched sqrt, DVE recip), scaled by 64 on Pool (tensor_scalar),
    cast bf16, PE-transposed, and packed to fp8e4 (TRN e4m3, max 240)
    wT8 [128e, 4chunk, 1024] blocks by the psum->sbuf ACT copy.
  * data ships natural (row norms only) and pre-transposed (dataT,
    f32), cast on device to fp8 dataT8 [128, 4chunk, 4096].  Data is
    NOT normalized pre-GEMM; 1/(64*||d_b||) lands at the epilogue (max
    over proxies commutes with positive per-batch-row scaling).
  * GEMM in MatmulPerfMode.DoubleRow (fp8e4, 0.5 cycles/row = 2x the
    bf16/f32r rate): each matmul contracts 256 e-dims (2 chunks x 128
    partitions), stationary dataT8 slice [128, 2, 128b], moving wT8
    slice [128, 2, 256w], out psum [128b, 256w].  8 matmuls fill one
    psum tile [128, 1024] (32 classes x 32 proxies x 128 batch rows);
    each psum window's accumulation group closes before the next opens
    (hw allows one pending group per zero region).
  * Per-class segment max: PSUM can only be read by single-PSUM-input
    DVE/ACT ops (GPSIMD cannot touch PSUM; TensorTensor allows just
    one PSUM operand), so 4/9 of tiles drain via DVE tensor_reduce
    directly and 5/9 via an ACT copy to bf16 SBUF followed by a DVE
    2x-mode tensor_tensor tree (32->16->8) + tensor_reduce (8->1).
    Pool (which in this toolchain runs only tensor_scalar/memset/
    affine_select/DMA) carries the w-normalize and epilogue muls.
  * DMAs are batched (w half-blocks, 8 m-tiles of data, 2 m-tiles of
    output per dma_start) because each dma_start holds the issuing
    sequencer for ~1.4us + transfer time, all on SP in priority order.
  * Software pipeline: block b+1's prep chain is emitted as per-tile
    thunks dripped between block b's m-tiles so the in-order ACT queue
    never head-blocks the psum-draining copies; block 0's m>=16 tiles
    are deferred past block 1 to hide the dataT second-half casts.
"""

import numpy as np

# Problem shapes (hardcoded; harness always calls with these).
B = 4096
E = 512
C = 2000
PROXIES = 32
P = C * PROXIES
N_CORES = 8
P_SHARD = P // N_CORES      # 8000 w rows per core
C_SHARD = C // N_CORES      # 250 classes per core
EPS = 1e-12
SW = 64.0                   # fp8 pre-scale for normalized w rows

PE_TILE = 128
KC = E // PE_TILE           # 4 contraction chunks of 128
MT = B // PE_TILE           # 32 batch m-tiles
WBLK = 1024                 # w rows per block (32 classes)
NG = 256                    # w rows per matmul moving tile
DIRECT_MOD, DIRECT_LT = 9, 5    # 5 of 9 tiles -> DVE-direct reduce;
                                # rest drain via ACT copy + Pool tree
EPI_B = 4                   # m-tiles per output DMA


def build_bass_kernel(b=B, e=E, p_shard=P_SHARD, proxies=PROXIES):
    from concourse import bacc, mybir, masks
    from concourse.tile import TileContext

    f32 = mybir.dt.float32
    bf16 = mybir.dt.bfloat16
    f8 = mybir.dt.float8e4
    AF = mybir.ActivationFunctionType
    AX = mybir.AxisListType
    OP = mybir.AluOpType
    DR = mybir.MatmulPerfMode.DoubleRow

    assert e == KC * PE_TILE and b == MT * PE_TILE
    c_shard = p_shard // proxies

    blocks = []
    rs = 0
    while rs < p_shard:
        blocks.append((rs, min(WBLK, p_shard - rs)))
        rs += WBLK

    nc = bacc.Bacc("TRN2", target_bir_lowering=False, debug=False)
    data_d = nc.dram_tensor("data", [b, e], f32, kind="ExternalInput")
    dataT_d = nc.dram_tensor("dataT", [e, b], f32, kind="ExternalInput")
    w_d = nc.dram_tensor("w", [p_shard, e], f32, kind="ExternalInput")
    out_d = nc.dram_tensor("out", [b, c_shard], f32, kind="ExternalOutput")

    # dram views for batched (multi-row-block) DMAs
    data_v = data_d[:, :].rearrange("(u p) e -> p u e", p=PE_TILE)  # [128,32,512]
    out_v = out_d[:, :].rearrange("(u p) c -> p u c", p=PE_TILE)    # [128,32,250]

    with TileContext(nc) as tc:
        with tc.tile_pool(name="sbuf", bufs=1) as sb, \
             tc.tile_pool(name="mmps", bufs=3, space="PSUM") as psm, \
             tc.tile_pool(name="trps", bufs=2, space="PSUM") as pst:

            identb = sb.tile([PE_TILE, PE_TILE], bf16, tag="identb")
            masks.make_identity(nc, identb[:])

            dataT8 = sb.tile([PE_TILE, KC, b], f8, tag="dataT8")
            dn_ssq = sb.tile([PE_TILE, MT], f32, tag="dn_ssq")
            dnorm64 = sb.tile([PE_TILE, MT], f32, tag="dnorm64")

            out3 = sb.tile([PE_TILE, MT, c_shard], bf16, tag="out3")

            # ---- dataT -> fp8 (casts on DVE; DMAs on SP).
            # Column halves so m-tiles 0..15 unblock before the full load;
            # DMA and cast emission are split so late-arriving halves never
            # head-block the DVE queue in front of prep norm ops.
            dt32s = {}
            QP = b // 4           # dataT column piece (quarter)

            def emit_dataT_dma(parts, eng=None):
                for p, k in parts:
                    cs = p * QP
                    dt32 = sb.tile([PE_TILE, QP], f32, tag="dt32",
                                   bufs=4, name="dt32")
                    (eng or nc.sync).dma_start(
                        dt32[:], dataT_d[k * 128:(k + 1) * 128,
                                         cs:cs + QP])
                    dt32s[(p, k)] = dt32

            def emit_dataT_cast(parts, eng="dve"):
                for p, k in parts:
                    cs = p * QP
                    src = dt32s.pop((p, k))
                    if eng == "dve":
                        nc.vector.tensor_copy(dataT8[:, k, cs:cs + QP],
                                              src)
                    else:
                        nc.scalar.copy(dataT8[:, k, cs:cs + QP], src)

            # ---- w block prep: batched DMA + normalize*64 -> fp8 -> T.
            # Returns (wT8, thunks): the DMAs issue immediately; the compute
            # chain comes back as per-tile thunks so the caller can drip them
            # between m-tiles — a monolithic prep chain in ACT's in-order
            # queue would head-block the psum-draining copies for ~12us at
            # every block boundary.
            def prep_block(blk):
                rs, nwb = blk
                nt = (nwb + PE_TILE - 1) // PE_TILE     # 128-row tiles (<=8)
                wblk = sb.tile([PE_TILE, 8, e], f32, tag="wblk", bufs=2,
                               name="wblk")
                # half-block DMAs (512 rows each; tail: 256 + 64)
                spans = []
                done = 0
                while done < nwb:
                    full = min(4, (nwb - done) // PE_TILE)
                    if full:
                        spans.append((done // PE_TILE, full, PE_TILE))
                        done += full * PE_TILE
                    else:
                        spans.append((done // PE_TILE, 1, nwb - done))
                        done = nwb
                for (ut, cnt, rows) in spans:
                    r0 = rs + ut * PE_TILE
                    if cnt > 1:
                        src = w_d[r0:r0 + cnt * PE_TILE, :].rearrange(
                            "(u p) e -> p u e", p=PE_TILE)
                        nc.sync.dma_start(wblk[:, ut:ut + cnt, :], src)
                    else:
                        nc.sync.dma_start(wblk[:rows, ut, :],
                                          w_d[r0:r0 + rows, :])

                wT8 = sb.tile([PE_TILE, KC, WBLK], f8, tag="wT8", bufs=3,
                              name="wT8")
                ssq8 = sb.tile([PE_TILE, 8], f32, tag="ssq8", bufs=2,
                               name="ssq8")
                if nwb % PE_TILE:
                    # tail tile writes <128 partitions; keep sqrt input finite
                    nc.vector.memset(ssq8[:], 1.0)
                nrm8 = sb.tile([PE_TILE, 8], f32, tag="nrm8", bufs=2,
                               name="nrm8")
                nmx8 = sb.tile([PE_TILE, 8], f32, tag="nmx8", bufs=2,
                               name="nmx8")
                r64 = sb.tile([PE_TILE, 8], f32, tag="r64", bufs=2,
                              name="r64")

                def sq_unit(t):
                    rows = min(PE_TILE, nwb - t * PE_TILE)
                    sq = sb.tile([PE_TILE, e], f32, tag="sq", bufs=4,
                                 name="sq")
                    nc.scalar.activation(sq[:rows], wblk[:rows, t, :],
                                         AF.Square,
                                         accum_out=ssq8[:rows, t:t + 1])

                def norm_unit(h0, h1):
                    nc.scalar.sqrt(nrm8[:, h0:h1], ssq8[:, h0:h1])
                    nc.vector.tensor_scalar(nmx8[:, h0:h1], nrm8[:, h0:h1],
                                            EPS, 1.0 / SW, op0=OP.max,
                                            op1=OP.mult)
                    nc.vector.reciprocal(r64[:, h0:h1], nmx8[:, h0:h1])

                def mtc_unit(t):
                    rows = min(PE_TILE, nwb - t * PE_TILE)
                    wn8 = sb.tile([PE_TILE, e], bf16, tag="wn8",
                                  bufs=8, name="wn8")
                    nc.gpsimd.tensor_scalar(wn8[:rows], wblk[:rows, t, :],
                                            r64[:rows, t:t + 1], None,
                                            op0=OP.mult)
                    trp = pst.tile([PE_TILE, KC * PE_TILE], bf16,
                                   tag="trp", name="trp")
                    for k in range(KC):
                        nc.tensor.transpose(
                            trp[:, k * 128: k * 128 + rows],
                            wn8[:rows, k * 128:(k + 1) * 128],
                            identb[:rows, :rows])
                    nc.scalar.copy(
                        wT8[:, :, t * PE_TILE: t * PE_TILE + rows],
                        trp[:].rearrange("p (k r) -> p k r",
                                         k=KC)[:, :, :rows])

                thunks = []
                for h0 in range(0, nt, 4):
                    h1 = min(h0 + 4, nt)
                    for t in range(h0, h1):
                        thunks.append(lambda t=t: sq_unit(t))
                    thunks.append(lambda h0=h0, h1=h1: norm_unit(h0, h1))
                    for t in range(h0, h1):
                        thunks.append(lambda t=t: mtc_unit(t))
                return wT8, thunks

            # ---- matmuls + segment max for one (block, m-tile) ----
            def mm_block(bi, blk, wT8, m_range=range(MT), interleave=None):
                rs, nwb = blk
                c0 = rs // proxies
                ncls = nwb // proxies
                groups = []
                go = 0
                while go < nwb:
                    groups.append((go, min(NG, nwb - go)))
                    go += NG
                for m in m_range:
                    if interleave is not None:
                        interleave(m)
                    ps = psm.tile([PE_TILE, WBLK], f32, tag="mmps",
                                  name="mmps")
                    col = m * PE_TILE
                    # j inner: each psum window's accumulation group closes
                    # before the next opens (hw allows one pending group per
                    # zero region)
                    for (go, gn) in groups:
                        for j in range(2):
                            nc.tensor.matmul(
                                ps[:, go:go + gn],
                                dataT8[:, 2 * j:2 * j + 2, col:col + 128],
                                wT8[:, 2 * j:2 * j + 2, go:go + gn],
                                start=(j == 0),
                                stop=(j == 1),
                                perf_mode=DR,
                            )
                    ps3 = ps[:, :nwb].rearrange("p (c g) -> p c g", g=proxies)

                    def epilogue(me):
                        u = me % EPI_B
                        if u == 0:
                            self_fin[0] = sb.tile(
                                [PE_TILE, EPI_B, c_shard], f32, tag="fin",
                                bufs=2, name="fin")
                        nc.gpsimd.tensor_scalar(
                            self_fin[0][:, u, :], out3[:, me, :],
                            dnorm64[:, me:me + 1], None, op0=OP.mult)
                        if u == EPI_B - 1:
                            q = me // EPI_B
                            eng = nc.sync if q % 2 == 0 else nc.scalar
                            eng.dma_start(
                                out_v[:, q * EPI_B:(q + 1) * EPI_B, :],
                                self_fin[0][:])

                    # PSUM exits only via single-PSUM-input ops on TRN2:
                    # [D,B,B,D] per 4 m-tiles — D drains by DVE tensor_reduce
                    # directly; adjacent B pairs share one bf16 staging tile
                    # (two ACT copies) and ONE fused 4D DVE max tree, which
                    # amortizes the tree instruction inits across both tiles.
                    if (bi == 0 and m < 8) or m % 4 in (0, 3):
                        nc.vector.tensor_reduce(out3[:, m, c0:c0 + ncls],
                                                ps3, axis=AX.X, op=OP.max)
                        if bi == len(blocks) - 1:
                            epilogue(m)
                    else:
                        pu = 0 if m % 4 == 1 else 1
                        if pu == 0:
                            self_t32[0] = sb.tile(
                                [PE_TILE, 2, 32, proxies], bf16,
                                tag="t32", bufs=4, name="t32")
                        t32 = self_t32[0]
                        nc.scalar.copy(t32[:, pu, :ncls, :], ps3)
                        if pu == 1:
                            t16 = sb.tile([PE_TILE, 2, 32, proxies // 2],
                                          bf16, tag="t16", bufs=4,
                                          name="t16")
                            nc.vector.tensor_tensor(
                                t16[:, :, :ncls, :],
                                t32[:, :, :ncls, :proxies // 2],
                                t32[:, :, :ncls, proxies // 2:], OP.max)
                            t8 = sb.tile([PE_TILE, 2, 32, proxies // 4],
                                         bf16, tag="t8", bufs=4, name="t8")
                            nc.vector.tensor_tensor(
                                t8[:, :, :ncls, :],
                                t16[:, :, :ncls, :proxies // 4],
                                t16[:, :, :ncls, proxies // 4:], OP.max)
                            nc.vector.tensor_reduce(
                                out3[:, m - 1:m + 1, c0:c0 + ncls],
                                t8[:, :, :ncls, :], axis=AX.X, op=OP.max)
                            if bi == len(blocks) - 1:
                                epilogue(m - 1)
                                epilogue(m)

            # ---- data row norms (DMA eager, squares dripped) ----
            def push_data_norms_q(q, out):
                dnat = sb.tile([PE_TILE, 8, e], f32, tag="dnat", bufs=2,
                               name="dnat")
                nc.sync.dma_start(dnat[:], data_v[:, q * 8:(q + 1) * 8, :])

                def square(u):
                    m = q * 8 + u
                    dsq = sb.tile([PE_TILE, e], f32, tag="dsq", bufs=4,
                                  name="dsq")
                    nc.scalar.activation(dsq[:], dnat[:, u, :], AF.Square,
                                         accum_out=dn_ssq[:, m:m + 1])
                for u in range(8):
                    out.append(lambda u=u: square(u))

            def emit_data_norms_fin():
                dnr = sb.tile([PE_TILE, MT], f32, tag="dnr", name="dnr")
                nc.scalar.sqrt(dnr[:], dn_ssq[:])
                dnx = sb.tile([PE_TILE, MT], f32, tag="dnx", name="dnx")
                nc.vector.tensor_scalar(dnx[:], dnr[:], EPS, SW,
                                        op0=OP.max, op1=OP.mult)
                nc.vector.reciprocal(dnorm64[:], dnx[:])

            # ---- main software pipeline ----
            # All DMAs issue from SP in priority order: dataT chunks 0,1 of
            # the first column half (unblock j=0), w block 0, dataT 2,3,
            # then the second column half (only needed from m-tile 16 on).
            # Block 0's m>=16 tiles are deferred into a makeup pass after
            # block 1 so the late second-half casts never stall the PE.
            self_fin = [None]
            self_t32 = [None]
            drip = []

            def drip_cb(m):
                n = 2 if len(drip) >= 20 else 1
                for _ in range(min(n, len(drip))):
                    drip.pop(0)()

            def h1_cast_piece(k, q):
                cs = (2 + q) * QP
                src = dt32s.pop((2 + q, k))
                nc.vector.tensor_copy(dataT8[:, k, cs:cs + QP], src)

            emit_dataT_dma([(0, 0), (0, 1)])
            emit_dataT_cast([(0, 0), (0, 1)])
            wT8s = [None] * len(blocks)
            wT8s[0], th0 = prep_block(blocks[0])
            for t in th0:
                t()                       # block 0 prep is the startup path
            emit_dataT_dma([(0, 2), (0, 3)])
            emit_dataT_cast([(0, 2), (0, 3)])
            emit_dataT_dma([(1, k) for k in range(KC)])
            # piece-1 casts drip through block 0 (m-tiles 8-15 need them)
            # ahead of block 1's prep thunks so they never head-block DVE
            drip.extend((lambda k=k: emit_dataT_cast([(1, k)]))
                        for k in range(KC))
            wT8s[1], th1 = prep_block(blocks[1])
            drip.extend(th1)              # dripped through block 0's m-tiles
            emit_dataT_dma([(p, k) for p in (2, 3) for k in range(KC)])

            for bi, blk in enumerate(blocks):
                if bi == 1:
                    drip.extend(
                        (lambda k=k, q=q: h1_cast_piece(k, q))
                        for k in range(KC) for q in range(2))
                if 1 <= bi < len(blocks) - 1:
                    wT8s[bi + 1], th = prep_block(blocks[bi + 1])
                    drip.extend(th)
                if 2 <= bi < 6:
                    push_data_norms_q(bi - 2, drip)
                mm_block(bi, blk, wT8s[bi],
                         range(16) if bi == 0 else range(MT),
                         interleave=drip_cb)
                if bi == 1:
                    mm_block(0, blocks[0], wT8s[0], range(16, MT),
                             interleave=drip_cb)
                while drip:               # flush leftovers at block end
                    drip.pop(0)()
                if bi == 6:
                    emit_data_norms_fin()

    nc.compile()
    return nc


_NC_CACHE = {}


def _get_nc(key, **kwargs):
    if key not in _NC_CACHE:
        _NC_CACHE[key] = build_bass_kernel(**kwargs)
    return _NC_CACHE[key]


def kernel(data, w1, segment_ids=None):
    """Full-input entry point: shards internally across 8 NeuronCores."""
    from concourse.bass_utils import run_bass_kernel_spmd

    data = np.ascontiguousarray(np.asarray(data), dtype=np.float32)
    w1 = np.ascontiguousarray(np.asarray(w1), dtype=np.float32)
    assert data.shape == (B, E) and w1.shape == (P, E)
    dataT = np.ascontiguousarray(data.T)

    nc = _get_nc("full")
    in_maps = [
        {"data": data, "dataT": dataT,
         "w": w1[i * P_SHARD:(i + 1) * P_SHARD]}
        for i in range(N_CORES)
    ]
    res = run_bass_kernel_spmd(nc, in_maps, core_ids=list(range(N_CORES)))
    out = np.empty((C, B), dtype=np.float32)
    for i in range(N_CORES):
        out[i * C_SHARD:(i + 1) * C_SHARD, :] = res.results[i]["out"].T
    return out


# revision 86
# speedup vs baseline: 1.0010x; 1.0010x over previous
"""Trainium2 Bass kernel for nn_ImprintedModel (retrieval_knn).

Computes y[c, b] = max over the 32 proxies p of class c of
    (w1[p] / ||w1[p]||) . (data[b] / ||data[b]||)
for data [4096, 512], w1 [64000, 512] (2000 classes x 32 proxies),
output [2000, 4096] fp32.

Sharding: w1 rows (and hence classes) split across 8 cores (8000 rows =
250 classes per core); data replicated. Each core computes its 250
output rows for all 4096 batch columns; host concatenates/transposes.

Device algorithm (fp8 fast path; measured rel err ~1.77e-2 < 2e-2):
  * w rows are L2-normalized on device (ACT square+accum, half-block
    batched sqrt, DVE recip), scaled by 64 on Pool (tensor_scalar),
    cast bf16, PE-transposed, and packed to fp8e4 (TRN e4m3, max 240)
    wT8 [128e, 4chunk, 1024] blocks by the psum->sbuf ACT copy.
  * data ships natural (row norms only) and pre-transposed (dataT,
    f32), cast on device to fp8 dataT8 [128, 4chunk, 4096].  Data is
    NOT normalized pre-GEMM; 1/(64*||d_b||) lands at the epilogue (max
    over proxies commutes with positive per-batch-row scaling).
  * GEMM in MatmulPerfMode.DoubleRow (fp8e4, 0.5 cycles/row = 2x the
    bf16/f32r rate): each matmul contracts 256 e-dims (2 chunks x 128
    partitions), stationary dataT8 slice [128, 2, 128b], moving wT8
    slice [128, 2, 256w], out psum [128b, 256w].  8 matmuls fill one
    psum tile [128, 1024] (32 classes x 32 proxies x 128 batch rows);
    each psum window's accumulation group closes before the next opens
    (hw allows one pending group per zero region).
  * Per-class segment max: PSUM can only be read by single-PSUM-input
    DVE/ACT ops (GPSIMD cannot touch PSUM; TensorTensor allows just
    one PSUM operand), so 4/9 of tiles drain via DVE tensor_reduce
    directly and 5/9 via an ACT copy to bf16 SBUF followed by a DVE
    2x-mode tensor_tensor tree (32->16->8) + tensor_reduce (8->1).
    Pool (which in this toolchain runs only tensor_scalar/memset/
    affine_select/DMA) carries the w-normalize and epilogue muls.
  * DMAs are batched (w half-blocks, 8 m-tiles of data, 2 m-tiles of
    output per dma_start) because each dma_start holds the issuing
    sequencer for ~1.4us + transfer time, all on SP in priority order.
  * Software pipeline: block b+1's prep chain is emitted as per-tile
    thunks dripped between block b's m-tiles so the in-order ACT queue
    never head-blocks the psum-draining copies; block 0's m>=16 tiles
    are deferred past block 1 to hide the dataT second-half casts.
"""

import numpy as np

# Problem shapes (hardcoded; harness always calls with these).
B = 4096
E = 512
C = 2000
PROXIES = 32
P = C * PROXIES
N_CORES = 8
P_SHARD = P // N_CORES      # 8000 w rows per core
C_SHARD = C // N_CORES      # 250 classes per core
EPS = 1e-12
SW = 64.0                   # fp8 pre-scale for normalized w rows

PE_TILE = 128
KC = E // PE_TILE           # 4 contraction chunks of 128
MT = B // PE_TILE           # 32 batch m-tiles
WBLK = 1024                 # w rows per block (32 classes)
NG = 256                    # w rows per matmul moving tile
DIRECT_MOD, DIRECT_LT = 9, 5    # 5 of 9 tiles -> DVE-direct reduce;
                                # rest drain via ACT copy + Pool tree
EPI_B = 4                   # m-tiles per output DMA


def build_bass_kernel(b=B, e=E, p_shard=P_SHARD, proxies=PROXIES):
    from concourse import bacc, mybir, masks
    from concourse.tile import TileContext

    f32 = mybir.dt.float32
    bf16 = mybir.dt.bfloat16
    f8 = mybir.dt.float8e4
    AF = mybir.ActivationFunctionType
    AX = mybir.AxisListType
    OP = mybir.AluOpType
    DR = mybir.MatmulPerfMode.DoubleRow

    assert e == KC * PE_TILE and b == MT * PE_TILE
    c_shard = p_shard // proxies

    blocks = []
    rs = 0
    while rs < p_shard:
        blocks.append((rs, min(WBLK, p_shard - rs)))
        rs += WBLK

    nc = bacc.Bacc("TRN2", target_bir_lowering=False, debug=False)
    data_d = nc.dram_tensor("data", [b, e], f32, kind="ExternalInput")
    dataT_d = nc.dram_tensor("dataT", [e, b], f32, kind="ExternalInput")
    w_d = nc.dram_tensor("w", [p_shard, e], f32, kind="ExternalInput")
    out_d = nc.dram_tensor("out", [b, c_shard], f32, kind="ExternalOutput")

    # dram views for batched (multi-row-block) DMAs
    data_v = data_d[:, :].rearrange("(u p) e -> p u e", p=PE_TILE)  # [128,32,512]
    out_v = out_d[:, :].rearrange("(u p) c -> p u c", p=PE_TILE)    # [128,32,250]

    with TileContext(nc) as tc:
        with tc.tile_pool(name="sbuf", bufs=1) as sb, \
             tc.tile_pool(name="mmps", bufs=3, space="PSUM") as psm, \
             tc.tile_pool(name="trps", bufs=2, space="PSUM") as pst:

            identb = sb.tile([PE_TILE, PE_TILE], bf16, tag="identb")
            masks.make_identity(nc, identb[:])

            dataT8 = sb.tile([PE_TILE, KC, b], f8, tag="dataT8")
            dn_ssq = sb.tile([PE_TILE, MT], f32, tag="dn_ssq")
            dnorm64 = sb.tile([PE_TILE, MT], f32, tag="dnorm64")

            out3 = sb.tile([PE_TILE, MT, c_shard], bf16, tag="out3")

            # ---- dataT -> fp8 (casts on DVE; DMAs on SP).
            # Column halves so m-tiles 0..15 unblock before the full load;
            # DMA and cast emission are split so late-arriving halves never
            # head-block the DVE queue in front of prep norm ops.
            dt32s = {}
            QP = b // 4           # dataT column piece (quarter)

            def emit_dataT_dma(parts, eng=None):
                for p, k in parts:
                    cs = p * QP
                    dt32 = sb.tile([PE_TILE, QP], f32, tag="dt32",
                                   bufs=4, name="dt32")
                    (eng or nc.sync).dma_start(
                        dt32[:], dataT_d[k * 128:(k + 1) * 128,
                                         cs:cs + QP])
                    dt32s[(p, k)] = dt32

            def emit_dataT_cast(parts, eng="dve"):
                for p, k in parts:
                    cs = p * QP
                    src = dt32s.pop((p, k))
                    if eng == "dve":
                        nc.vector.tensor_copy(dataT8[:, k, cs:cs + QP],
                                              src)
                    else:
                        nc.scalar.copy(dataT8[:, k, cs:cs + QP], src)

            # ---- w block prep: batched DMA + normalize*64 -> fp8 -> T.
            # Returns (wT8, thunks): the DMAs issue immediately; the compute
            # chain comes back as per-tile thunks so the caller can drip them
            # between m-tiles — a monolithic prep chain in ACT's in-order
            # queue would head-block the psum-draining copies for ~12us at
            # every block boundary.
            def prep_block(blk):
                rs, nwb = blk
                nt = (nwb + PE_TILE - 1) // PE_TILE     # 128-row tiles (<=8)
                wblk = sb.tile([PE_TILE, 8, e], f32, tag="wblk", bufs=2,
                               name="wblk")
                # half-block DMAs (512 rows each; tail: 256 + 64)
                spans = []
                done = 0
                while done < nwb:
                    full = min(4, (nwb - done) // PE_TILE)
                    if full:
                        spans.append((done // PE_TILE, full, PE_TILE))
                        done += full * PE_TILE
                    else:
                        spans.append((done // PE_TILE, 1, nwb - done))
                        done = nwb
                for (ut, cnt, rows) in spans:
                    r0 = rs + ut * PE_TILE
                    if cnt > 1:
                        src = w_d[r0:r0 + cnt * PE_TILE, :].rearrange(
                            "(u p) e -> p u e", p=PE_TILE)
                        nc.sync.dma_start(wblk[:, ut:ut + cnt, :], src)
                    else:
                        nc.sync.dma_start(wblk[:rows, ut, :],
                                          w_d[r0:r0 + rows, :])

                wT8 = sb.tile([PE_TILE, KC, WBLK], f8, tag="wT8", bufs=3,
                              name="wT8")
                ssq8 = sb.tile([PE_TILE, 8], f32, tag="ssq8", bufs=2,
                               name="ssq8")
                if nwb % PE_TILE:
                    # tail tile writes <128 partitions; keep sqrt input finite
                    nc.vector.memset(ssq8[:], 1.0)
                nrm8 = sb.tile([PE_TILE, 8], f32, tag="nrm8", bufs=2,
                               name="nrm8")
                nmx8 = sb.tile([PE_TILE, 8], f32, tag="nmx8", bufs=2,
                               name="nmx8")
                r64 = sb.tile([PE_TILE, 8], f32, tag="r64", bufs=2,
                              name="r64")

                def sq_unit(t):
                    rows = min(PE_TILE, nwb - t * PE_TILE)
                    sq = sb.tile([PE_TILE, e], f32, tag="sq", bufs=4,
                                 name="sq")
                    nc.scalar.activation(sq[:rows], wblk[:rows, t, :],
                                         AF.Square,
                                         accum_out=ssq8[:rows, t:t + 1])

                def norm_unit(h0, h1):
                    nc.scalar.sqrt(nrm8[:, h0:h1], ssq8[:, h0:h1])
                    nc.vector.tensor_scalar(nmx8[:, h0:h1], nrm8[:, h0:h1],
                                            EPS, 1.0 / SW, op0=OP.max,
                                            op1=OP.mult)
                    nc.vector.reciprocal(r64[:, h0:h1], nmx8[:, h0:h1])

                def mtc_unit(t):
                    rows = min(PE_TILE, nwb - t * PE_TILE)
                    wn8 = sb.tile([PE_TILE, e], bf16, tag="wn8",
                                  bufs=8, name="wn8")
                    nc.gpsimd.tensor_scalar(wn8[:rows], wblk[:rows, t, :],
                                            r64[:rows, t:t + 1], None,
                                            op0=OP.mult)
                    trp = pst.tile([PE_TILE, KC * PE_TILE], bf16,
                                   tag="trp", name="trp")
                    for k in range(KC):
                        nc.tensor.transpose(
                            trp[:, k * 128: k * 128 + rows],
                            wn8[:rows, k * 128:(k + 1) * 128],
                            identb[:rows, :rows])
                    nc.scalar.copy(
                        wT8[:, :, t * PE_TILE: t * PE_TILE + rows],
                        trp[:].rearrange("p (k r) -> p k r",
                                         k=KC)[:, :, :rows])

                thunks = []
                for h0 in range(0, nt, 4):
                    h1 = min(h0 + 4, nt)
                    for t in range(h0, h1):
                        thunks.append(lambda t=t: sq_unit(t))
                    thunks.append(lambda h0=h0, h1=h1: norm_unit(h0, h1))
                    for t in range(h0, h1):
                        thunks.append(lambda t=t: mtc_unit(t))
                return wT8, thunks

            # ---- matmuls + segment max for one (block, m-tile) ----
            def mm_block(bi, blk, wT8, m_range=range(MT), interleave=None):
                rs, nwb = blk
                c0 = rs // proxies
                ncls = nwb // proxies
                groups = []
                go = 0
                while go < nwb:
                    groups.append((go, min(NG, nwb - go)))
                    go += NG
                for m in m_range:
                    if interleave is not None:
                        interleave(m)
                    ps = psm.tile([PE_TILE, WBLK], f32, tag="mmps",
                                  name="mmps")
                    col = m * PE_TILE
                    # j inner: each psum window's accumulation group closes
                    # before the next opens (hw allows one pending group per
                    # zero region)
                    for (go, gn) in groups:
                        for j in range(2):
                            nc.tensor.matmul(
                                ps[:, go:go + gn],
                                dataT8[:, 2 * j:2 * j + 2, col:col + 128],
                                wT8[:, 2 * j:2 * j + 2, go:go + gn],
                                start=(j == 0),
                                stop=(j == 1),
                                perf_mode=DR,
                            )
                    ps3 = ps[:, :nwb].rearrange("p (c g) -> p c g", g=proxies)

                    def epilogue(me):
                        u = me % EPI_B
                        if u == 0:
                            self_fin[0] = sb.tile(
                                [PE_TILE, EPI_B, c_shard], f32, tag="fin",
                                bufs=2, name="fin")
                        nc.gpsimd.tensor_scalar(
                            self_fin[0][:, u, :], out3[:, me, :],
                            dnorm64[:, me:me + 1], None, op0=OP.mult)
                        if u == EPI_B - 1:
                            q = me // EPI_B
                            eng = nc.sync if q % 2 == 0 else nc.scalar
                            eng.dma_start(
                                out_v[:, q * EPI_B:(q + 1) * EPI_B, :],
                                self_fin[0][:])

                    # PSUM exits only via single-PSUM-input ops on TRN2:
                    # [D,B,B,D] per 4 m-tiles — D drains by DVE tensor_reduce
                    # directly; adjacent B pairs share one bf16 staging tile
                    # (two ACT copies) and ONE fused 4D DVE max tree, which
                    # amortizes the tree instruction inits across both tiles.
                    if (bi == 0 and m < 8) or m % 4 in (0, 3):
                        nc.vector.tensor_reduce(out3[:, m, c0:c0 + ncls],
                                                ps3, axis=AX.X, op=OP.max)
                        if bi == len(blocks) - 1:
                            epilogue(m)
                    else:
                        pu = 0 if m % 4 == 1 else 1
                        if pu == 0:
                            self_t32[0] = sb.tile(
                                [PE_TILE, 2, 32, proxies], bf16,
                                tag="t32", bufs=6, name="t32")
                        t32 = self_t32[0]
                        nc.scalar.copy(t32[:, pu, :ncls, :], ps3)
                        if pu == 1:
                            t16 = sb.tile([PE_TILE, 2, 32, proxies // 2],
                                          bf16, tag="t16", bufs=6,
                                          name="t16")
                            nc.vector.tensor_tensor(
                                t16[:, :, :ncls, :],
                                t32[:, :, :ncls, :proxies // 2],
                                t32[:, :, :ncls, proxies // 2:], OP.max)
                            t8 = sb.tile([PE_TILE, 2, 32, proxies // 4],
                                         bf16, tag="t8", bufs=6, name="t8")
                            nc.vector.tensor_tensor(
                                t8[:, :, :ncls, :],
                                t16[:, :, :ncls, :proxies // 4],
                                t16[:, :, :ncls, proxies // 4:], OP.max)
                            nc.vector.tensor_reduce(
                                out3[:, m - 1:m + 1, c0:c0 + ncls],
                                t8[:, :, :ncls, :], axis=AX.X, op=OP.max)
                            if bi == len(blocks) - 1:
                                epilogue(m - 1)
                                epilogue(m)

            # ---- data row norms (DMA eager, squares dripped) ----
            def push_data_norms_q(q, out):
                dnat = sb.tile([PE_TILE, 8, e], f32, tag="dnat", bufs=2,
                               name="dnat")
                nc.sync.dma_start(dnat[:], data_v[:, q * 8:(q + 1) * 8, :])

                def square(u):
                    m = q * 8 + u
                    dsq = sb.tile([PE_TILE, e], f32, tag="dsq", bufs=4,
                                  name="dsq")
                    nc.scalar.activation(dsq[:], dnat[:, u, :], AF.Square,
                                         accum_out=dn_ssq[:, m:m + 1])
                for u in range(8):
                    out.append(lambda u=u: square(u))

            def emit_data_norms_fin():
                dnr = sb.tile([PE_TILE, MT], f32, tag="dnr", name="dnr")
                nc.scalar.sqrt(dnr[:], dn_ssq[:])
                dnx = sb.tile([PE_TILE, MT], f32, tag="dnx", name="dnx")
                nc.vector.tensor_scalar(dnx[:], dnr[:], EPS, SW,
                                        op0=OP.max, op1=OP.mult)
                nc.vector.reciprocal(dnorm64[:], dnx[:])

            # ---- main software pipeline ----
            # All DMAs issue from SP in priority order: dataT chunks 0,1 of
            # the first column half (unblock j=0), w block 0, dataT 2,3,
            # then the second column half (only needed from m-tile 16 on).
            # Block 0's m>=16 tiles are deferred into a makeup pass after
            # block 1 so the late second-half casts never stall the PE.
            self_fin = [None]
            self_t32 = [None]
            drip = []

            def drip_cb(m):
                if drip:
                    drip.pop(0)()

            def h1_cast_piece(k, q):
                cs = (2 + q) * QP
                src = dt32s.pop((2 + q, k))
                nc.vector.tensor_copy(dataT8[:, k, cs:cs + QP], src)

            emit_dataT_dma([(0, 0), (0, 1)])
            emit_dataT_cast([(0, 0), (0, 1)])
            wT8s = [None] * len(blocks)
            wT8s[0], th0 = prep_block(blocks[0])
            for t in th0:
                t()                       # block 0 prep is the startup path
            emit_dataT_dma([(0, 2), (0, 3)])
            emit_dataT_cast([(0, 2), (0, 3)])
            emit_dataT_dma([(1, k) for k in range(KC)])
            # piece-1 casts drip through block 0 (m-tiles 8-15 need them)
            # ahead of block 1's prep thunks so they never head-block DVE
            drip.extend((lambda k=k: emit_dataT_cast([(1, k)]))
                        for k in range(KC))
            wT8s[1], th1 = prep_block(blocks[1])
            drip.extend(th1)              # dripped through block 0's m-tiles
            emit_dataT_dma([(p, k) for p in (2, 3) for k in range(KC)])

            for bi, blk in enumerate(blocks):
                if bi == 1:
                    drip.extend(
                        (lambda k=k, q=q: h1_cast_piece(k, q))
                        for k in range(KC) for q in range(2))
                if 1 <= bi < len(blocks) - 1:
                    wT8s[bi + 1], th = prep_block(blocks[bi + 1])
                    drip.extend(th)
                if 2 <= bi < 6:
                    push_data_norms_q(bi - 2, drip)
                mm_block(bi, blk, wT8s[bi],
                         range(16) if bi == 0 else range(MT),
                         interleave=drip_cb)
                if bi == 1:
                    mm_block(0, blocks[0], wT8s[0], range(16, MT),
                             interleave=drip_cb)
                while drip:               # flush leftovers at block end
                    drip.pop(0)()
                if bi == 6:
                    emit_data_norms_fin()

    nc.compile()
    return nc


_NC_CACHE = {}


def _get_nc(key, **kwargs):
    if key not in _NC_CACHE:
        _NC_CACHE[key] = build_bass_kernel(**kwargs)
    return _NC_CACHE[key]


def kernel(data, w1, segment_ids=None):
    """Full-input entry point: shards internally across 8 NeuronCores."""
    from concourse.bass_utils import run_bass_kernel_spmd

    data = np.ascontiguousarray(np.asarray(data), dtype=np.float32)
    w1 = np.ascontiguousarray(np.asarray(w1), dtype=np.float32)
    assert data.shape == (B, E) and w1.shape == (P, E)
    dataT = np.ascontiguousarray(data.T)

    nc = _get_nc("full")
    in_maps = [
        {"data": data, "dataT": dataT,
         "w": w1[i * P_SHARD:(i + 1) * P_SHARD]}
        for i in range(N_CORES)
    ]
    res = run_bass_kernel_spmd(nc, in_maps, core_ids=list(range(N_CORES)))
    out = np.empty((C, B), dtype=np.float32)
    for i in range(N_CORES):
        out[i * C_SHARD:(i + 1) * C_SHARD, :] = res.results[i]["out"].T
    return out
